# revision 66
# baseline (speedup 1.0000x reference)
"""Trainium2 Bass kernel for causal self-attention (RoPE + per-head RMSNorm), v3.

Reference computation (B=2, T=2048, C=1024, H=16, D=64):
    q = rope(rmsnorm(x @ Wq.T)); k = rope(rmsnorm(x @ Wk.T)); v = x @ Wv.T
    out = softmax(causal(q k^T / sqrt(D))) v @ Wo.T

Sharding over 8 NeuronCores: core c -> batch b = c//4, head-group g = c%4
(4 heads = 256 features per group).  Feature-major ("transposed") on-chip
layout; softmax denominator via a ones column appended to V.

v3 changes vs v2 (trace-driven; ~347us -> ~300us):
  - input head was byte-bound: causal mask generated on-chip via
    affine_select (-1MB), xt loads split into token-block chunks so the
    first projections start after ~1.5MB instead of ~4.5MB, wo queues
    behind everything on the same ring (bandwidth priority by FIFO),
    tiny tables on the gpsimd ring
  - rsqrt for RMSNorm via ACT ln+exp (2 ops, natural_log_exp_and_others
    table set shared with the attention exp) instead of a 7-op DVE
    bit-trick chain; chain_b multiplies q by the PSUM broadcast directly;
    the sum-of-squares squares run on DVE from the bf16 copies (ACT's
    ~75%-busy window gates the attention exp stream)
  - softmax normalization no longer touches ACT (ACT is the bottleneck
    engine during attention); note reciprocal_approx_fast must NOT read
    PSUM directly (garbage results) -- a DVE copy stages the denominator
  - causal mask muls trimmed to the affected column range per r-tile
  - A2A bounce DMAs issued per token block right after each norm, off
    the critical path to the collective trigger, split over 2 HWDGE rings
  - PE keep-warm chain through the second AllToAll: the HAM clock gate
    re-throttles the PE to 1.2GHz after ~3.4us of idle, which made the
    post-collective o_proj run at half clock.  A serial MM->DVE-copy
    chain (~213ns MM every ~1.3us) on a dedicated PSUM bank keeps the
    activity window non-empty; it is pinned into the collective window
    by seeding it from the last-written y block (the Tile scheduler
    hoists it otherwise), and kept in its own tile (sharing an open
    o_proj accumulation tile serializes behind the stop matmul)
  - o_proj: even-mt phase during the collective, then co-outer odd-mt
    phase with per-co stores; two columns run as deferred full
    contractions on the first-freed PSUM rings
  (fp8 for the exchange payload + Wo was tried and REVERTED: o_proj is a
  zero-mean reduction, so fp8's 6% element noise does not average out;
  measured rel_err 3.8e-2 vs the 2e-2 gate.)
"""

import os
import sys

for _p in ("/opt/trn_rl_repo", "/root/.axon_site/_ro/trn_rl_repo"):
    if os.path.isdir(_p) and _p not in sys.path:
        sys.path.insert(0, _p)

import numpy as np
import ml_dtypes

import concourse.bass as bass
from concourse import bacc
import concourse.tile as tile
import concourse.mybir as mybir

BF16 = mybir.dt.bfloat16
FP8 = mybir.dt.float8e4
F32 = mybir.dt.float32
AF = mybir.ActivationFunctionType

B, T, C, H, D = 2, 2048, 1024, 16, 64
N_CORES = 8
GH = 4  # heads per core
GF = GH * D  # features per core (256)
TB = 512  # token block (matmul N)
KT = C // 128  # 8 contraction k-tiles
EPS = float(np.finfo(np.float32).eps)
ROPE_BASE = 10000.0


def build_nc(t=T):
    ntb = t // TB  # tq blocks
    ntt = t // 128  # token 128-tiles
    tsl = t // 4  # per-core token slice for o_proj

    nc = bacc.Bacc("TRN2", target_bir_lowering=False, debug=False, num_devices=N_CORES)

    xt = nc.dram_tensor("xt", [C, t], BF16, kind="ExternalInput")
    wq = nc.dram_tensor("wq", [C, GF], BF16, kind="ExternalInput")
    wk = nc.dram_tensor("wk", [C, GF], BF16, kind="ExternalInput")
    wv = nc.dram_tensor("wv", [C, GF], BF16, kind="ExternalInput")
    wo = nc.dram_tensor("wo", [C, C], BF16, kind="ExternalInput")
    cosf = nc.dram_tensor("cosf", [128, t], BF16, kind="ExternalInput")
    sinf = nc.dram_tensor("sinf", [128, t], BF16, kind="ExternalInput")
    pswap = nc.dram_tensor("pswap", [128, 128], BF16, kind="ExternalInput")
    blk2q = nc.dram_tensor("blk2q", [128, 2], BF16, kind="ExternalInput")
    blk2k = nc.dram_tensor("blk2k", [128, 2], BF16, kind="ExternalInput")
    out = nc.dram_tensor("out", [C, tsl], F32, kind="ExternalOutput")

    with tile.TileContext(nc) as tc:
        with (
            nc.allow_low_precision(reason="bf16 compute by design"),
            tc.tile_pool(name="p_xt", bufs=4 * KT) as p_xt,
            tc.tile_pool(name="p_w", bufs=1) as p_w,
            tc.tile_pool(name="p_wo", bufs=1) as p_wo,
            tc.tile_pool(name="p_tab", bufs=1) as p_tab,
            tc.tile_pool(name="p_qk", bufs=2) as p_qk,
            tc.tile_pool(name="p_v", bufs=ntt) as p_v,
            tc.tile_pool(name="p_y", bufs=2) as p_y,
            tc.tile_pool(name="p_yg", bufs=2 * KT) as p_yg,
            tc.tile_pool(name="p_pt", bufs=6) as p_pt,
            tc.tile_pool(name="p_tmp", bufs=2) as p_tmp,
            tc.tile_pool(name="p_pq", bufs=2, space="PSUM") as p_pq,
            tc.tile_pool(name="p_po", bufs=2, space="PSUM") as p_po,
            tc.tile_pool(name="p_sc", bufs=2, space="PSUM") as p_sc,
            tc.tile_pool(name="p_dram", bufs=2, space="DRAM") as p_dram,
        ):
            # ---- input DMAs ---------------------------------------------
            # The input stream is byte-bound (~290GB/s once the ~13us runtime
            # barrier clears).  One HWDGE ring carries everything in priority
            # order (FIFO per ring = bandwidth priority): proj j0 needs
            # wq+wk+xt chunk 0; cos/sin by the first chain_b; wv by the first
            # emit_v; wo only at o_proj.  Tiny tables ride the gpsimd ring.
            wq_fl = p_w.tile([128, KT * GF], BF16, tag="wq", name="wq_fl")
            nc.sync.dma_start(
                wq_fl[:].rearrange("p (c f) -> p c f", c=KT),
                wq[:].rearrange("(c p) f -> p c f", p=128),
            )
            wk_fl = p_w.tile([128, KT * GF], BF16, tag="wk", name="wk_fl")
            nc.sync.dma_start(
                wk_fl[:].rearrange("p (c f) -> p c f", c=KT),
                wk[:].rearrange("(c p) f -> p c f", p=128),
            )
            # xt loads in token-block chunks: proj/attention for block j only
            # need chunk j, so compute starts after 1MB instead of 4MB.
            xt_sb = [[None] * ntb for _ in range(KT)]

            def load_xt_chunk(jc):
                for ct in range(KT):
                    x_t = p_xt.tile([128, TB], BF16, tag="xt", name=f"xt{ct}_{jc}")
                    nc.sync.dma_start(
                        x_t[:], xt[ct * 128 : (ct + 1) * 128, jc * TB : (jc + 1) * TB]
                    )
                    xt_sb[ct][jc] = x_t

            load_xt_chunk(0)
            wv_fl = p_w.tile([128, KT * GF], BF16, tag="wv", name="wv_fl")
            nc.sync.dma_start(
                wv_fl[:].rearrange("p (c f) -> p c f", c=KT),
                wv[:].rearrange("(c p) f -> p c f", p=128),
            )
            load_xt_chunk(1)
            cos_sb = p_tab.tile([128, t], BF16, tag="cos")
            nc.sync.dma_start(cos_sb[:], cosf[:])
            sin_sb = p_tab.tile([128, t], BF16, tag="sin")
            nc.sync.dma_start(sin_sb[:], sinf[:])
            load_xt_chunk(2)
            load_xt_chunk(3)
            wo_fl = p_wo.tile([128, KT * C], BF16, tag="wo", name="wo_fl")
            nc.sync.dma_start(
                wo_fl[:].rearrange("p (c f) -> p c f", c=KT),
                wo[:].rearrange("(c p) f -> p c f", p=128),
            )
            blk2q_sb = p_tab.tile([128, 2], BF16, tag="blk2q")
            nc.gpsimd.dma_start(blk2q_sb[:], blk2q[:])
            blk2k_sb = p_tab.tile([128, 2], BF16, tag="blk2k")
            nc.gpsimd.dma_start(blk2k_sb[:], blk2k[:])
            pswap_sb = p_tab.tile([128, 128], BF16, tag="pswap")
            nc.gpsimd.dma_start(pswap_sb[:], pswap[:])
            # causal mask tiles, generated on-chip: mask[p, r*2TB + h*TB + c]
            # = 1.0 iff c >= 128*r + p (keep), else 0.0; identical per head h.
            mask_sb = p_tab.tile([128, 4 * 2 * TB], BF16, tag="mask")
            nc.gpsimd.memset(mask_sb[:], 1.0)
            for r in range(4):
                reg = mask_sb[:, r * 2 * TB : (r + 1) * 2 * TB].rearrange(
                    "p (h c) -> p h c", h=2
                )
                nc.gpsimd.affine_select(
                    out=reg,
                    in_=reg,
                    pattern=[[0, 2], [1, TB]],
                    compare_op=mybir.AluOpType.is_ge,
                    fill=0.0,
                    base=-128 * r,
                    channel_multiplier=-1,
                )
            ones64 = p_tab.tile([97, 64], BF16, tag="ones64")
            nc.vector.memset(ones64[:], 1.0)
            eps_sb = p_tab.tile([97, 1], F32, tag="eps")
            nc.vector.memset(eps_sb[:], EPS)
            # dummy activation at t~0: forces the ~2.7us exp/ln table load
            # to overlap the input-DMA head instead of the first real Ln.
            actw = p_tab.tile([1, 1], BF16, tag="actw")
            nc.scalar.activation(actw[:], eps_sb[0:1, 0:1], AF.Exp)

            qh_sb = [p_qk.tile([128, t], BF16, tag="qk0", name="qh0"),
                     p_qk.tile([128, t], BF16, tag="qk1", name="qh1")]
            kh_sb = [p_qk.tile([128, t], BF16, tag="qk0", name="kh0"),
                     p_qk.tile([128, t], BF16, tag="qk1", name="kh1")]
            v_sb = [None] * ntt

            # ---- PE keep-warm machinery ---------------------------------
            # The HAM re-throttles the PE to 1.2GHz after a ~3.4us idle
            # window and takes ~3.4us of sustained work to recover, so the
            # o_proj matmuls after the second AllToAll would run at half
            # clock.  A serial MM->DVE-copy->MM chain (one ~213ns MM every
            # ~1.3us) keeps every HAM window non-empty through the
            # collective at negligible cost.  The seed copy reads data only
            # ready at the start of the dead span, pinning the chain there
            # (the Tile scheduler would otherwise hoist it earlier).
            def emit_keepwarm(n, target, name, seed):
                cur = p_tmp.tile([1, TB], BF16, tag="kwc", name=f"kw{name}s")
                nc.vector.tensor_copy(cur[:], seed)
                for i in range(n):
                    nc.tensor.matmul(target[0:1, 0:TB], ones64[0:1, 0:1],
                                     cur[:1, :], start=True, stop=True)
                    nxt = p_tmp.tile([1, TB], BF16, tag="kwc", name=f"kw{name}{i}")
                    nc.vector.tensor_copy(nxt[:], target[0:1, 0:TB])
                    cur = nxt

            # ---- building blocks ----------------------------------------
            def emit_proj(w_fl, mt, j):
                """q/k projection block: pq[feat128, TB] for token block j."""
                pq = p_pq.tile([128, TB], F32, tag="pq", name=f"pq{mt}_{j}")
                for ct in range(KT):
                    o = ct * GF + mt * 128
                    nc.tensor.matmul(
                        pq[:],
                        w_fl[:, o : o + 128],
                        xt_sb[ct][j][:],
                        start=(ct == 0),
                        stop=(ct == KT - 1),
                    )
                return pq

            def chain_a2(pq_q, pq_k, name):
                """paired q+k sum-of-squares; inv = rsqrt(ms) via ACT ln+exp.

                The two pss results land col-packed in one PSUM bank (rows
                0/32 for q, 64/96 for k); rsqrt(ms) = exp(-0.5*ln(ms/D+EPS))
                runs as 2 ACT ops on [97, TB] in the same
                natural_log_exp_and_others table set as the attention exp.
                """
                # squares on DVE from the bf16 copies: keeps ACT free for the
                # exp stream (its ~75%-busy window gates attention) and
                # shortens the pq PSUM lifetime to a single consumer.
                qb_q = p_tmp.tile([128, TB], BF16, tag="qbq", name=f"qbq_{name}")
                nc.vector.tensor_copy(qb_q[:], pq_q[:])
                qb_k = p_tmp.tile([128, TB], BF16, tag="qbk", name=f"qbk_{name}")
                nc.vector.tensor_copy(qb_k[:], pq_k[:])
                sq_q = p_tmp.tile([128, TB], BF16, tag="sq", name=f"sqq_{name}")
                nc.vector.tensor_mul(sq_q[:], qb_q[:], qb_q[:])
                sq_k = p_tmp.tile([128, TB], BF16, tag="sqk", name=f"sqk_{name}")
                nc.vector.tensor_mul(sq_k[:], qb_k[:], qb_k[:])
                pss = p_pq.tile([128, TB], F32, tag="pq", name=f"pss_{name}")
                nc.tensor.matmul(pss[0:1, :], blk2q_sb[:, 0:1], sq_q[:], start=True,
                                 stop=True, tile_position=(0, 0))
                nc.tensor.matmul(pss[32:33, :], blk2q_sb[:, 1:2], sq_q[:],
                                 start=True, stop=True, tile_position=(0, 32))
                nc.tensor.matmul(pss[64:65, :], blk2k_sb[:, 0:1], sq_k[:],
                                 start=True, stop=True, tile_position=(0, 64))
                nc.tensor.matmul(pss[96:97, :], blk2k_sb[:, 1:2], sq_k[:],
                                 start=True, stop=True, tile_position=(0, 96))
                lnms = p_tmp.tile([97, TB], F32, tag="lnms", name=f"ln_{name}")
                nc.scalar.activation(
                    lnms[:], pss[0:97, :], AF.Ln, bias=eps_sb[:], scale=1.0 / D
                )
                inv = p_tmp.tile([97, TB], BF16, tag="inv", name=f"inv_{name}")
                nc.scalar.activation(inv[:], lnms[:], AF.Exp, scale=-0.5)
                return qb_q, qb_k, inv

            def chain_b(qb, inv_t, is_k, dst, jb, name):
                """broadcast inv over head rows, apply, rope, write dst[:, jb]."""
                r0 = 64 if is_k else 0
                pinvb = p_pq.tile([128, TB], F32, tag="pq", name=f"pinvb_{name}")
                nc.tensor.matmul(pinvb[0:64, :], ones64[r0 : r0 + 1, :],
                                 inv_t[r0 : r0 + 1, :], start=True, stop=True,
                                 tile_position=(r0, 0))
                nc.tensor.matmul(pinvb[64:128, :], ones64[r0 + 32 : r0 + 33, :],
                                 inv_t[r0 + 32 : r0 + 33, :], start=True,
                                 stop=True, tile_position=(r0 + 32, 64))
                qn = p_tmp.tile([128, TB], BF16, tag="qn", name=f"qn_{name}")
                nc.vector.tensor_mul(qn[:], qb[:], pinvb[:])
                pqs = p_pq.tile([128, TB], F32, tag="pq", name=f"pqs_{name}")
                nc.tensor.matmul(pqs[:], pswap_sb[:], qn[:], start=True, stop=True)
                t1 = p_tmp.tile([128, TB], BF16, tag="t1", name=f"t1_{name}")
                nc.gpsimd.tensor_mul(t1[:], qn[:], cos_sb[:, jb])
                t2 = p_tmp.tile([128, TB], BF16, tag="t2", name=f"t2_{name}")
                nc.vector.tensor_mul(t2[:], pqs[:], sin_sb[:, jb])
                nc.gpsimd.tensor_add(dst[:, jb], t1[:], t2[:])

            def emit_v(tt):
                pv = p_pq.tile([128, TB], F32, tag="pq", name=f"pv{tt}")
                jc, tc_ = tt // 4, (tt % 4) * 128
                for ct in range(KT):
                    nc.tensor.matmul(
                        pv[:, 0:GF],
                        xt_sb[ct][jc][:, tc_ : tc_ + 128],
                        wv_fl[:, ct * GF : (ct + 1) * GF],
                        start=(ct == 0),
                        stop=(ct == KT - 1),
                    )
                v_t = p_v.tile([128, GH * (D + 1)], BF16, tag="v", name=f"v{tt}")
                vsrc = pv[:, 0:GF].rearrange("p (h d) -> p h d", h=GH)
                vdst = v_t[:].rearrange("p (h d) -> p h d", h=GH, d=D + 1)
                nc.vector.tensor_copy(vdst[:, :, 0:D], vsrc)
                nc.vector.memset(vdst[:, :, D : D + 1], 1.0)
                v_sb[tt] = v_t

            # ---- filler machinery ---------------------------------------
            fill_q = []

            def fill(n=1):
                for _ in range(n):
                    if not fill_q:
                        return
                    fill_q.pop(0)()

            def drain_fill():
                while fill_q:
                    fill_q.pop(0)()

            # ---- head: q/k mt0 projections + paired chains, v0..3 -------
            # per j: proj q, proj k, paired rsqrt; chain_b lags one j so the
            # PE never sits directly behind the ACT/DVE chain.  v-tile pairs
            # land mid-head so their DVE casts are ready for attention j0.
            recs = {}
            vq = [0, 1, 2, 3]
            for j in range(ntb):
                pq_q = emit_proj(wq_fl, 0, j)
                pq_k = emit_proj(wk_fl, 0, j)
                recs[j] = chain_a2(pq_q, pq_k, f"h0_{j}")
                # chain_b(j-1) after proj(j): by the time the PE reaches its
                # matmuls the DVE rsqrt for j-1 is long done (no stall).
                if j > 0:
                    qb0, kb0, inv0 = recs[j - 1]
                    jb0 = slice((j - 1) * TB, j * TB)
                    chain_b(qb0, inv0, False, qh_sb[0], jb0, f"q0_{j-1}")
                    chain_b(kb0, inv0, True, kh_sb[0], jb0, f"k0_{j-1}")
                if j >= 2:
                    emit_v(vq.pop(0))
                    emit_v(vq.pop(0))
            qb0, kb0, inv0 = recs[ntb - 1]
            jb0 = slice((ntb - 1) * TB, ntb * TB)
            chain_b(qb0, inv0, False, qh_sb[0], jb0, f"q0_{ntb-1}")
            chain_b(kb0, inv0, True, kh_sb[0], jb0, f"k0_{ntb-1}")
            while vq:
                emit_v(vq.pop(0))

            # ---- filler for attention phase -----------------------------
            # order: mt1 paired projections+chains per j, then v per deadline
            def mk_proj_chain(j):
                st = {}

                def do_a():
                    pq = emit_proj(wq_fl, 1, j)
                    pk = emit_proj(wk_fl, 1, j)
                    st["r"] = chain_a2(pq, pk, f"h1_{j}")

                def do_bq():
                    chain_b(st["r"][0], st["r"][2], False, qh_sb[1],
                            slice(j * TB, (j + 1) * TB), f"q1_{j}")

                def do_bk():
                    chain_b(st["r"][1], st["r"][2], True, kh_sb[1],
                            slice(j * TB, (j + 1) * TB), f"k1_{j}")

                return do_a, do_bq, do_bk

            for j in range(ntb):
                do_a, do_bq, do_bk = mk_proj_chain(j)
                fill_q.extend([do_a, do_bq, do_bk])
                if j < 3:
                    for tt in range(4 * (j + 1), 4 * (j + 2)):
                        fill_q.append(lambda tt=tt: emit_v(tt))

            # ---- attention ----------------------------------------------
            def norm(hp, j, po, y_t):
                # ACT-free: ACT is the bottleneck engine during attention.
                jb = slice(j * TB, (j + 1) * TB)
                rec = []
                for hl in range(2):
                    dn = p_tmp.tile([1, TB], F32, tag=f"den{hl}", name=f"dn{hl}")
                    nc.vector.tensor_copy(dn[:], po[hl][64:65, :])
                    rf = p_tmp.tile([1, TB], F32, tag=f"recf{hl}", name=f"rf{hl}")
                    nc.vector.reciprocal_approx_fast(out=rf[:], in_=dn[:])
                    rc = p_tmp.tile([1, TB], BF16, tag=f"rec{hl}", name=f"rc{hl}")
                    nc.vector.tensor_copy(rc[:], rf[:])
                    rec.append(rc)
                pr = p_pq.tile([128, TB], F32, tag="pq", name="pr")
                nc.tensor.matmul(
                    pr[0:64, :], ones64[0:1, :], rec[0][:], start=True, stop=True,
                    tile_position=(0, 0),
                )
                nc.tensor.matmul(
                    pr[64:128, :], ones64[0:1, :], rec[1][:], start=True, stop=True,
                    tile_position=(0, 64),
                )
                r_sb = p_tmp.tile([128, TB], BF16, tag="rsb", name="r_sb")
                nc.vector.tensor_copy(r_sb[:], pr[:])
                nc.vector.tensor_mul(y_t[0:64, jb], po[0][0:64, :], r_sb[0:64, :])
                nc.vector.tensor_mul(y_t[64:128, jb], po[1][0:64, :], r_sb[64:128, :])

            # partition-id-derived base for the valid A2A blocks (b = pid//4)
            pid = nc.sync.partition_id()
            base = nc.s_assert_within((pid >> 2) * 4, 0, 4, skip_runtime_assert=True)
            yg_sb = [None] * KT
            y_ts = []
            bounce_in = []
            bounce_out = []
            fill_ctr = 0
            for hp in range(2):
                y_t = p_y.tile([128, t], BF16, tag="y", name=f"y{hp}")
                y_ts.append(y_t)
                # exchange payload in fp8 (o_proj re-averages over 1024
                # inputs, so the 3-bit mantissa noise washes out) and the
                # bounce DMA for token block j fires as soon as its norm
                # lands, off the critical path to the collective trigger.
                bin_t = p_dram.tile([8 * 128, tsl], BF16, tag=f"bin{hp}")
                bout_t = p_dram.tile([8 * 128, tsl], BF16, tag=f"bout{hp}")
                bounce_in.append(bin_t)
                bounce_out.append(bout_t)
                for j in range(ntb):
                    jb = slice(j * TB, (j + 1) * TB)
                    po = [
                        p_po.tile([D + 1, TB], F32, tag="po", name=f"po{hl}")
                        for hl in range(2)
                    ]
                    n_tt = 4 * (j + 1)
                    pend = []
                    for tt in range(n_tt):
                        # scores for both heads -> one 2-bank psum tile
                        sc = p_sc.tile([128, 2 * TB], F32, tag="sc", name="sc")
                        for hl in range(2):
                            hofs = hl * 64
                            nc.tensor.matmul(
                                sc[:, hl * TB : (hl + 1) * TB],
                                kh_sb[hp][hofs : hofs + 64, tt * 128 : (tt + 1) * 128],
                                qh_sb[hp][hofs : hofs + 64, jb],
                                start=True,
                                stop=True,
                                tile_position=(hofs, 0),
                            )
                        pt = p_pt.tile([128, 2 * TB], BF16, tag="pt", name="pt")
                        nc.scalar.activation(
                            pt[:], sc[:], AF.Exp, scale=1.0 / np.sqrt(D)
                        )
                        r = tt - 4 * j
                        if r >= 0:  # diagonal tile: causal mask, both heads.
                            # only cols < 128*(r+1) of each head half can be
                            # masked; trim the mul to that range.
                            w = 128 * (r + 1)
                            pt3 = pt[:].rearrange("p (h c) -> p h c", h=2)
                            mk3 = mask_sb[
                                :, r * 2 * TB : (r + 1) * 2 * TB
                            ].rearrange("p (h c) -> p h c", h=2)
                            nc.vector.tensor_mul(
                                pt3[:, :, 0:w], pt3[:, :, 0:w], mk3[:, :, 0:w]
                            )
                        pend.append((tt, pt))
                        if len(pend) > 2:
                            att_tt, att_pt = pend.pop(0)
                            for hl in range(2):
                                h = 2 * hp + hl
                                nc.tensor.matmul(
                                    po[hl][:],
                                    v_sb[att_tt][:, h * (D + 1) : (h + 1) * (D + 1)],
                                    att_pt[:, hl * TB : (hl + 1) * TB],
                                    start=(att_tt == 0),
                                    stop=(att_tt == n_tt - 1),
                                )
                        # pace fillers ~2-per-3 steps through hp0 so PE work
                        # remains for the late j blocks (an exhausted queue
                        # left j3-boundary gaps that re-throttle the HAM);
                        # deadlines still hold: each v tile lands >=4 steps
                        # before its first attention consumer.
                        if hp == 1:
                            fill(1)
                        else:
                            fill_ctr += 1
                            if fill_ctr % 3 != 0:
                                fill(1)
                    for att_tt, att_pt in pend:
                        for hl in range(2):
                            h = 2 * hp + hl
                            nc.tensor.matmul(
                                po[hl][:],
                                v_sb[att_tt][:, h * (D + 1) : (h + 1) * (D + 1)],
                                att_pt[:, hl * TB : (hl + 1) * TB],
                                start=(att_tt == 0),
                                stop=(att_tt == n_tt - 1),
                            )
                    norm(hp, j, po, y_t)
                    for s in (j, j + 4):
                        eng = nc.sync if s < 4 else nc.scalar
                        eng.dma_start(
                            bin_t[s * 128 : (s + 1) * 128, :],
                            y_t[:, j * tsl : (j + 1) * tsl],
                        )

                # 8-way AllToAll (4-core groups unsupported): shard s carries
                # our features(hp) for token block s%4.
                nc.gpsimd.collective_compute(
                    "AllToAll",
                    mybir.AluOpType.bypass,
                    ins=[bin_t.opt()],
                    outs=[bout_t.opt()],
                    replica_groups=[[0, 1, 2, 3, 4, 5, 6, 7]],
                )
                for gi in range(4):
                    yg_t = p_yg.tile([128, tsl], BF16, tag="yg", name=f"yg{hp}_{gi}")
                    nc.sync.dma_start(
                        yg_t[:], bout_t[bass.ts(base + gi, 128), :]
                    )
                    yg_sb[2 * gi + hp] = yg_t

            drain_fill()

            # ---- gather valid A2A blocks (dynamic: depends on batch) ----
            # our batch's ranks are 4b..4b+3 where b = partition_id // 4.
            # yg[2*gi + hp] = features [256*gi + 128*hp, +128) of our tokens.
            tc.tile_set_cur_wait(50)  # push o_proj to the schedule end

            # ---- o_proj: out^T[cout, tsl], junk-free 8-mt contraction ----
            # mt-outer so each phase starts the moment its first yg block
            # lands.  Even mts (hp0, from AllToAll #1) accumulate during the
            # second AllToAll; a keep-warm chain paces the PE through the
            # collective so the odd-mt phase runs at full clock.  co7 runs
            # as one deferred full contraction, freeing its PSUM half-bank
            # as the keep-warm target.
            def wo_sl(mt, co):
                return wo_fl[:, mt * C + co * 128 : mt * C + co * 128 + 128]

            pouts = []
            for co in range(6):
                if co < 2:
                    pout = p_po.tile([128, TB], F32, tag="po", name=f"oo{co}")
                elif co < 4:
                    pout = p_pq.tile([128, TB], F32, tag="pq", name=f"oo{co}")
                elif co == 4:
                    sc2 = p_sc.tile([128, 2 * TB], F32, tag="sc", name=f"oo{co}")
                    pout = sc2[:, 0:TB]
                else:
                    pout = sc2[:, TB : 2 * TB]
                pouts.append(pout)
            # dedicated keep-warm bank: sharing a tile with an open o_proj
            # accumulation would serialize the chain behind the stop matmul
            # (tile-granular dependency tracking).
            kw_tail = p_sc.tile([128, 2 * TB], F32, tag="sc", name="kw_tail")
            for n_mt, mt in enumerate((0, 2, 4, 6)):
                for co in range(6):
                    nc.tensor.matmul(
                        pouts[co][:], wo_sl(mt, co), yg_sb[mt][:],
                        start=(n_mt == 0), stop=False,
                    )
            tc.tile_set_cur_wait(51)  # keep-warm strictly between the phases
            emit_keepwarm(
                20, kw_tail, "t", y_ts[1][0:1, (ntb - 1) * TB : ntb * TB]
            )
            tc.tile_set_cur_wait(52)

            def store(co, pout):
                o_sb = p_tmp.tile([128, tsl], F32, tag=f"osb{co % 2}", name="o_sb")
                if co % 2 == 0:
                    nc.vector.tensor_copy(o_sb[:], pout[:])
                else:
                    nc.scalar.copy(o_sb[:], pout[:])
                deng = nc.sync if co % 2 == 0 else nc.scalar
                deng.dma_start(out[co * 128 : (co + 1) * 128, :], o_sb[:])

            # co-outer so each co's store fires the moment its contraction
            # stops; co6/co7 (deferred full contractions -- their banks
            # hosted the keep-warm chain) land on the po/pq rings, which
            # free first (right after co0/co2's stores).
            for co in range(6):
                for n_mt, mt in enumerate((1, 3, 5, 7)):
                    nc.tensor.matmul(
                        pouts[co][:], wo_sl(mt, co), yg_sb[mt][:],
                        start=False, stop=(n_mt == 3),
                    )
                store(co, pouts[co])
            for co in (6, 7):
                pool = p_po if co == 6 else p_pq
                pd = pool.tile([128, TB], F32, tag="po" if co == 6 else "pq",
                               name=f"oo{co}")
                for mt in range(KT):
                    nc.tensor.matmul(
                        pd[:], wo_sl(mt, co), yg_sb[mt][:],
                        start=(mt == 0), stop=(mt == KT - 1),
                    )
                store(co, pd)

    nc.compile()
    return nc


# ---------------------------------------------------------------------------
# host side
# ---------------------------------------------------------------------------


def _rope_tables(t):
    inv_freq = 1.0 / (ROPE_BASE ** (np.arange(0, D, 2, dtype=np.float64) / D))  # [32]
    ang = np.arange(t, dtype=np.float64)[:, None] * inv_freq[None, :]  # [t, 32]
    cos = np.cos(ang).astype(np.float32)
    sin = np.sin(ang).astype(np.float32)
    cosf = np.empty((128, t), np.float32)
    sinf = np.empty((128, t), np.float32)
    for r in range(128):
        d = r % 64
        f = d if d < 32 else d - 32
        cosf[r] = cos[:, f]
        sinf[r] = -sin[:, f] if d < 32 else sin[:, f]
    return cosf, sinf


def _consts(t):
    cosf, sinf = _rope_tables(t)
    pswap = np.zeros((128, 128), np.float32)
    for j in range(128):
        d = j % 64
        i = (j - 32) if d >= 32 else (j + 32)
        pswap[i, j] = 1.0
    maskt = np.zeros((128, 4 * 2 * TB), np.float32)
    for r in range(4):
        for p in range(128):
            lo = 128 * r + p
            if lo < TB:
                maskt[p, r * 2 * TB + lo : r * 2 * TB + TB] = 1.0
                maskt[p, r * 2 * TB + TB + lo : (r + 1) * 2 * TB] = 1.0
    return cosf, sinf, pswap, maskt


def _blk2w(w):
    # blk2 with 1/w^2 weights: pss then sums (w*q)^2 / w^2 = q^2, so the
    # rsqrt sees the unscaled mean-square while Wq rows carry the w factor.
    wf = np.where(w == 0.0, 1.0, w.astype(np.float64))
    e = np.zeros((128, 2), np.float32)
    e[0:64, 0] = 1.0 / wf[0:64] ** 2
    e[64:128, 1] = 1.0 / (wf[0:64] if len(w) == 64 else wf[64:128]) ** 2
    return e


def _bf(x):
    return np.ascontiguousarray(x).astype(ml_dtypes.bfloat16)


def make_in_maps(x, Wq, Wk, Wv, Wo, qn_w, kn_w, t=T):
    cosf, sinf, pswap, maskt = _consts(t)
    wscale_q = np.tile(np.asarray(qn_w, np.float32), H)
    wscale_k = np.tile(np.asarray(kn_w, np.float32), H)
    Wq = Wq * wscale_q[:, None]
    Wk = Wk * wscale_k[:, None]
    common = {
        "cosf": _bf(cosf),
        "sinf": _bf(sinf),
        "pswap": _bf(pswap),
        "blk2q": _bf(_blk2w(qn_w)),
        "blk2k": _bf(_blk2w(kn_w)),
    }
    wot = _bf(Wo.T)  # [c_in, c_out]
    in_maps = []
    for c in range(N_CORES):
        b, g = c // 4, c % 4
        fs = slice(GF * g, GF * (g + 1))
        in_maps.append(
            dict(
                common,
                xt=_bf(x[b, :t, :].T),
                wq=_bf(Wq[fs, :].T),
                wk=_bf(Wk[fs, :].T),
                wv=_bf(Wv[fs, :].T),
                wo=wot,
            )
        )
    return in_maps


def assemble(results, t=T):
    tsl = t // 4
    out = np.empty((B, t, C), np.float32)
    for c in range(N_CORES):
        b, g = c // 4, c % 4
        out[b, g * tsl : (g + 1) * tsl, :] = results[c]["out"].T
    return out


# -- cached PJRT runner (compile once, reuse across kernel() calls) ---------

_RUNNER = {}


def _get_runner(t=T):
    if t in _RUNNER:
        return _RUNNER[t]
    import jax
    from jax.sharding import Mesh, PartitionSpec
    from jax.experimental.shard_map import shard_map
    from concourse import bass2jax

    nc = build_nc(t)
    bass2jax.install_neuronx_cc_hook()

    partition_name = nc.partition_id_tensor.name if nc.partition_id_tensor else None
    in_names = []
    out_names = []
    out_avals = []
    zero_outs = []
    for alloc in nc.m.functions[0].allocations:
        if not isinstance(alloc, mybir.MemoryLocationSet):
            continue
        name = alloc.memorylocations[0].name
        if alloc.kind == "ExternalInput":
            if name == partition_name:
                continue
            in_names.append(name)
        elif alloc.kind == "ExternalOutput":
            shape = tuple(alloc.tensor_shape)
            dtype = mybir.dt.np(alloc.dtype)
            out_names.append(name)
            out_avals.append(jax.core.ShapedArray(shape, dtype))
            zero_outs.append(np.zeros(shape, dtype))
    n_params = len(in_names)
    all_names = in_names + out_names
    if partition_name is not None:
        all_names = all_names + [partition_name]

    def _body(*args):
        operands = list(args)
        if partition_name is not None:
            operands.append(bass2jax.partition_id_tensor())
        outs = bass2jax._bass_exec_p.bind(
            *operands,
            out_avals=tuple(out_avals),
            in_names=tuple(all_names),
            out_names=tuple(out_names),
            lowering_input_output_aliases=(),
            sim_require_finite=True,
            sim_require_nnan=True,
            nc=nc,
        )
        return tuple(outs)

    devices = jax.devices()[:N_CORES]
    mesh = Mesh(np.asarray(devices), ("core",))
    fn = jax.jit(
        shard_map(
            _body,
            mesh=mesh,
            in_specs=(PartitionSpec("core"),) * (n_params + len(out_names)),
            out_specs=(PartitionSpec("core"),) * len(out_names),
            check_rep=False,
        ),
        keep_unused=True,
    )
    runner = {
        "fn": fn,
        "body": _body,
        "in_names": in_names,
        "out_names": out_names,
        "out_avals": out_avals,
        "zero_outs": zero_outs,
        "jax": jax,
    }
    _RUNNER[t] = runner
    return runner


def run_device(in_maps, t=T):
    r = _get_runner(t)
    concat_in = [
        np.concatenate([np.asarray(m[name]) for m in in_maps], axis=0)
        for name in r["in_names"]
    ]
    concat_zero = [
        np.zeros((N_CORES * z.shape[0], *z.shape[1:]), z.dtype) for z in r["zero_outs"]
    ]
    outs = r["fn"](*concat_in, *concat_zero)
    results = []
    for c in range(N_CORES):
        results.append(
            {
                name: np.asarray(outs[i]).reshape(N_CORES, *r["out_avals"][i].shape)[c]
                for i, name in enumerate(r["out_names"])
            }
        )
    return results


def kernel(x, Wq, Wk, Wv, Wo, qn_w, kn_w):
    x = np.asarray(x, np.float32)
    in_maps = make_in_maps(
        x,
        np.asarray(Wq, np.float32),
        np.asarray(Wk, np.float32),
        np.asarray(Wv, np.float32),
        np.asarray(Wo, np.float32),
        np.asarray(qn_w, np.float32),
        np.asarray(kn_w, np.float32),
    )
    results = run_device(in_maps)
    return assemble(results)



# revision 68
# speedup vs baseline: 1.0011x; 1.0011x over previous
"""Trainium2 Bass kernel for causal self-attention (RoPE + per-head RMSNorm), v3.

Reference computation (B=2, T=2048, C=1024, H=16, D=64):
    q = rope(rmsnorm(x @ Wq.T)); k = rope(rmsnorm(x @ Wk.T)); v = x @ Wv.T
    out = softmax(causal(q k^T / sqrt(D))) v @ Wo.T

Sharding over 8 NeuronCores: core c -> batch b = c//4, head-group g = c%4
(4 heads = 256 features per group).  Feature-major ("transposed") on-chip
layout; softmax denominator via a ones column appended to V.

v3 changes vs v2 (trace-driven; ~347us -> ~300us):
  - input head was byte-bound: causal mask generated on-chip via
    affine_select (-1MB), xt loads split into token-block chunks so the
    first projections start after ~1.5MB instead of ~4.5MB, wo queues
    behind everything on the same ring (bandwidth priority by FIFO),
    tiny tables on the gpsimd ring
  - rsqrt for RMSNorm via ACT ln+exp (2 ops, natural_log_exp_and_others
    table set shared with the attention exp) instead of a 7-op DVE
    bit-trick chain; chain_b multiplies q by the PSUM broadcast directly;
    the sum-of-squares squares run on DVE from the bf16 copies (ACT's
    ~75%-busy window gates the attention exp stream)
  - softmax normalization no longer touches ACT (ACT is the bottleneck
    engine during attention); note reciprocal_approx_fast must NOT read
    PSUM directly (garbage results) -- a DVE copy stages the denominator
  - causal mask muls trimmed to the affected column range per r-tile
  - A2A bounce DMAs issued per token block right after each norm, off
    the critical path to the collective trigger, split over 2 HWDGE rings
  - PE keep-warm chain through the second AllToAll: the HAM clock gate
    re-throttles the PE to 1.2GHz after ~3.4us of idle, which made the
    post-collective o_proj run at half clock.  A serial MM->DVE-copy
    chain (~213ns MM every ~1.3us) on a dedicated PSUM bank keeps the
    activity window non-empty; it is pinned into the collective window
    by seeding it from the last-written y block (the Tile scheduler
    hoists it otherwise), and kept in its own tile (sharing an open
    o_proj accumulation tile serializes behind the stop matmul)
  - o_proj: even-mt phase during the collective, then co-outer odd-mt
    phase with per-co stores; two columns run as deferred full
    contractions on the first-freed PSUM rings
  (fp8 for the exchange payload + Wo was tried and REVERTED: o_proj is a
  zero-mean reduction, so fp8's 6% element noise does not average out;
  measured rel_err 3.8e-2 vs the 2e-2 gate.)
"""

import os
import sys

for _p in ("/opt/trn_rl_repo", "/root/.axon_site/_ro/trn_rl_repo"):
    if os.path.isdir(_p) and _p not in sys.path:
        sys.path.insert(0, _p)

import numpy as np
import ml_dtypes

import concourse.bass as bass
from concourse import bacc
import concourse.tile as tile
import concourse.mybir as mybir

BF16 = mybir.dt.bfloat16
FP8 = mybir.dt.float8e4
F32 = mybir.dt.float32
AF = mybir.ActivationFunctionType

B, T, C, H, D = 2, 2048, 1024, 16, 64
N_CORES = 8
GH = 4  # heads per core
GF = GH * D  # features per core (256)
TB = 512  # token block (matmul N)
KT = C // 128  # 8 contraction k-tiles
EPS = float(np.finfo(np.float32).eps)
ROPE_BASE = 10000.0


def build_nc(t=T):
    ntb = t // TB  # tq blocks
    ntt = t // 128  # token 128-tiles
    tsl = t // 4  # per-core token slice for o_proj

    nc = bacc.Bacc("TRN2", target_bir_lowering=False, debug=False, num_devices=N_CORES)

    xt = nc.dram_tensor("xt", [C, t], BF16, kind="ExternalInput")
    wq = nc.dram_tensor("wq", [C, GF], BF16, kind="ExternalInput")
    wk = nc.dram_tensor("wk", [C, GF], BF16, kind="ExternalInput")
    wv = nc.dram_tensor("wv", [C, GF], BF16, kind="ExternalInput")
    wo = nc.dram_tensor("wo", [C, C], BF16, kind="ExternalInput")
    cosf = nc.dram_tensor("cosf", [128, t], BF16, kind="ExternalInput")
    sinf = nc.dram_tensor("sinf", [128, t], BF16, kind="ExternalInput")
    pswap = nc.dram_tensor("pswap", [128, 128], BF16, kind="ExternalInput")
    blk2q = nc.dram_tensor("blk2q", [128, 2], BF16, kind="ExternalInput")
    blk2k = nc.dram_tensor("blk2k", [128, 2], BF16, kind="ExternalInput")
    out = nc.dram_tensor("out", [C, tsl], F32, kind="ExternalOutput")

    with tile.TileContext(nc) as tc:
        with (
            nc.allow_low_precision(reason="bf16 compute by design"),
            tc.tile_pool(name="p_xt", bufs=4 * KT) as p_xt,
            tc.tile_pool(name="p_w", bufs=1) as p_w,
            tc.tile_pool(name="p_wo", bufs=1) as p_wo,
            tc.tile_pool(name="p_tab", bufs=1) as p_tab,
            tc.tile_pool(name="p_qk", bufs=2) as p_qk,
            tc.tile_pool(name="p_v", bufs=ntt) as p_v,
            tc.tile_pool(name="p_y", bufs=2) as p_y,
            tc.tile_pool(name="p_yg", bufs=2 * KT) as p_yg,
            tc.tile_pool(name="p_pt", bufs=6) as p_pt,
            tc.tile_pool(name="p_tmp", bufs=2) as p_tmp,
            tc.tile_pool(name="p_pq", bufs=2, space="PSUM") as p_pq,
            tc.tile_pool(name="p_po", bufs=2, space="PSUM") as p_po,
            tc.tile_pool(name="p_sc", bufs=2, space="PSUM") as p_sc,
            tc.tile_pool(name="p_dram", bufs=2, space="DRAM") as p_dram,
        ):
            # ---- input DMAs ---------------------------------------------
            # The input stream is byte-bound (~290GB/s once the ~13us runtime
            # barrier clears).  One HWDGE ring carries everything in priority
            # order (FIFO per ring = bandwidth priority): proj j0 needs
            # wq+wk+xt chunk 0; cos/sin by the first chain_b; wv by the first
            # emit_v; wo only at o_proj.  Tiny tables ride the gpsimd ring.
            wq_fl = p_w.tile([128, KT * GF], BF16, tag="wq", name="wq_fl")
            nc.sync.dma_start(
                wq_fl[:].rearrange("p (c f) -> p c f", c=KT),
                wq[:].rearrange("(c p) f -> p c f", p=128),
            )
            wk_fl = p_w.tile([128, KT * GF], BF16, tag="wk", name="wk_fl")
            nc.sync.dma_start(
                wk_fl[:].rearrange("p (c f) -> p c f", c=KT),
                wk[:].rearrange("(c p) f -> p c f", p=128),
            )
            # xt loads in token-block chunks: proj/attention for block j only
            # need chunk j, so compute starts after 1MB instead of 4MB.
            xt_sb = [[None] * ntb for _ in range(KT)]

            def load_xt_chunk(jc):
                for ct in range(KT):
                    x_t = p_xt.tile([128, TB], BF16, tag="xt", name=f"xt{ct}_{jc}")
                    nc.sync.dma_start(
                        x_t[:], xt[ct * 128 : (ct + 1) * 128, jc * TB : (jc + 1) * TB]
                    )
                    xt_sb[ct][jc] = x_t

            load_xt_chunk(0)
            wv_fl = p_w.tile([128, KT * GF], BF16, tag="wv", name="wv_fl")
            nc.sync.dma_start(
                wv_fl[:].rearrange("p (c f) -> p c f", c=KT),
                wv[:].rearrange("(c p) f -> p c f", p=128),
            )
            load_xt_chunk(1)
            cos_sb = p_tab.tile([128, t], BF16, tag="cos")
            nc.sync.dma_start(cos_sb[:], cosf[:])
            sin_sb = p_tab.tile([128, t], BF16, tag="sin")
            nc.sync.dma_start(sin_sb[:], sinf[:])
            load_xt_chunk(2)
            load_xt_chunk(3)
            wo_fl = p_wo.tile([128, KT * C], BF16, tag="wo", name="wo_fl")
            nc.sync.dma_start(
                wo_fl[:].rearrange("p (c f) -> p c f", c=KT),
                wo[:].rearrange("(c p) f -> p c f", p=128),
            )
            blk2q_sb = p_tab.tile([128, 2], BF16, tag="blk2q")
            nc.gpsimd.dma_start(blk2q_sb[:], blk2q[:])
            blk2k_sb = p_tab.tile([128, 2], BF16, tag="blk2k")
            nc.gpsimd.dma_start(blk2k_sb[:], blk2k[:])
            pswap_sb = p_tab.tile([128, 128], BF16, tag="pswap")
            nc.gpsimd.dma_start(pswap_sb[:], pswap[:])
            # causal mask tiles, generated on-chip: mask[p, r*2TB + h*TB + c]
            # = 1.0 iff c >= 128*r + p (keep), else 0.0; identical per head h.
            mask_sb = p_tab.tile([128, 4 * 2 * TB], BF16, tag="mask")
            nc.gpsimd.memset(mask_sb[:], 1.0)
            for r in range(4):
                reg = mask_sb[:, r * 2 * TB : (r + 1) * 2 * TB].rearrange(
                    "p (h c) -> p h c", h=2
                )
                nc.gpsimd.affine_select(
                    out=reg,
                    in_=reg,
                    pattern=[[0, 2], [1, TB]],
                    compare_op=mybir.AluOpType.is_ge,
                    fill=0.0,
                    base=-128 * r,
                    channel_multiplier=-1,
                )
            ones64 = p_tab.tile([97, 64], BF16, tag="ones64")
            nc.vector.memset(ones64[:], 1.0)
            eps_sb = p_tab.tile([97, 1], F32, tag="eps")
            nc.vector.memset(eps_sb[:], EPS)
            # dummy activation at t~0: forces the ~2.7us exp/ln table load
            # to overlap the input-DMA head instead of the first real Ln.
            actw = p_tab.tile([1, 1], BF16, tag="actw")
            nc.scalar.activation(actw[:], eps_sb[0:1, 0:1], AF.Exp)

            qh_sb = [p_qk.tile([128, t], BF16, tag="qk0", name="qh0"),
                     p_qk.tile([128, t], BF16, tag="qk1", name="qh1")]
            kh_sb = [p_qk.tile([128, t], BF16, tag="qk0", name="kh0"),
                     p_qk.tile([128, t], BF16, tag="qk1", name="kh1")]
            v_sb = [None] * ntt

            # ---- PE keep-warm machinery ---------------------------------
            # The HAM re-throttles the PE to 1.2GHz after a ~3.4us idle
            # window and takes ~3.4us of sustained work to recover, so the
            # o_proj matmuls after the second AllToAll would run at half
            # clock.  A serial MM->DVE-copy->MM chain (one ~213ns MM every
            # ~1.3us) keeps every HAM window non-empty through the
            # collective at negligible cost.  The seed copy reads data only
            # ready at the start of the dead span, pinning the chain there
            # (the Tile scheduler would otherwise hoist it earlier).
            def emit_keepwarm(n, target, name, seed):
                cur = p_tmp.tile([1, TB], BF16, tag="kwc", name=f"kw{name}s")
                nc.vector.tensor_copy(cur[:], seed)
                for i in range(n):
                    nc.tensor.matmul(target[0:1, 0:TB], ones64[0:1, 0:1],
                                     cur[:1, :], start=True, stop=True)
                    nxt = p_tmp.tile([1, TB], BF16, tag="kwc", name=f"kw{name}{i}")
                    nc.vector.tensor_copy(nxt[:], target[0:1, 0:TB])
                    cur = nxt

            # ---- building blocks ----------------------------------------
            def emit_proj(w_fl, mt, j):
                """q/k projection block: pq[feat128, TB] for token block j."""
                pq = p_pq.tile([128, TB], F32, tag="pq", name=f"pq{mt}_{j}")
                for ct in range(KT):
                    o = ct * GF + mt * 128
                    nc.tensor.matmul(
                        pq[:],
                        w_fl[:, o : o + 128],
                        xt_sb[ct][j][:],
                        start=(ct == 0),
                        stop=(ct == KT - 1),
                    )
                return pq

            def chain_a2(pq_q, pq_k, name):
                """paired q+k sum-of-squares; inv = rsqrt(ms) via ACT ln+exp.

                The two pss results land col-packed in one PSUM bank (rows
                0/32 for q, 64/96 for k); rsqrt(ms) = exp(-0.5*ln(ms/D+EPS))
                runs as 2 ACT ops on [97, TB] in the same
                natural_log_exp_and_others table set as the attention exp.
                """
                # squares on DVE from the bf16 copies: keeps ACT free for the
                # exp stream (its ~75%-busy window gates attention) and
                # shortens the pq PSUM lifetime to a single consumer.
                qb_q = p_tmp.tile([128, TB], BF16, tag="qbq", name=f"qbq_{name}")
                nc.vector.tensor_copy(qb_q[:], pq_q[:])
                qb_k = p_tmp.tile([128, TB], BF16, tag="qbk", name=f"qbk_{name}")
                nc.vector.tensor_copy(qb_k[:], pq_k[:])
                sq_q = p_tmp.tile([128, TB], BF16, tag="sq", name=f"sqq_{name}")
                nc.vector.tensor_mul(sq_q[:], qb_q[:], qb_q[:])
                sq_k = p_tmp.tile([128, TB], BF16, tag="sqk", name=f"sqk_{name}")
                nc.vector.tensor_mul(sq_k[:], qb_k[:], qb_k[:])
                pss = p_pq.tile([128, TB], F32, tag="pq", name=f"pss_{name}")
                nc.tensor.matmul(pss[0:1, :], blk2q_sb[:, 0:1], sq_q[:], start=True,
                                 stop=True, tile_position=(0, 0))
                nc.tensor.matmul(pss[32:33, :], blk2q_sb[:, 1:2], sq_q[:],
                                 start=True, stop=True, tile_position=(0, 32))
                nc.tensor.matmul(pss[64:65, :], blk2k_sb[:, 0:1], sq_k[:],
                                 start=True, stop=True, tile_position=(0, 64))
                nc.tensor.matmul(pss[96:97, :], blk2k_sb[:, 1:2], sq_k[:],
                                 start=True, stop=True, tile_position=(0, 96))
                lnms = p_tmp.tile([97, TB], F32, tag="lnms", name=f"ln_{name}")
                nc.scalar.activation(
                    lnms[:], pss[0:97, :], AF.Ln, bias=eps_sb[:], scale=1.0 / D
                )
                inv = p_tmp.tile([97, TB], BF16, tag="inv", name=f"inv_{name}")
                nc.scalar.activation(inv[:], lnms[:], AF.Exp, scale=-0.5)
                return qb_q, qb_k, inv

            def chain_b(qb, inv_t, is_k, dst, jb, name):
                """broadcast inv over head rows, apply, rope, write dst[:, jb]."""
                r0 = 64 if is_k else 0
                pinvb = p_pq.tile([128, TB], F32, tag="pq", name=f"pinvb_{name}")
                nc.tensor.matmul(pinvb[0:64, :], ones64[r0 : r0 + 1, :],
                                 inv_t[r0 : r0 + 1, :], start=True, stop=True,
                                 tile_position=(r0, 0))
                nc.tensor.matmul(pinvb[64:128, :], ones64[r0 + 32 : r0 + 33, :],
                                 inv_t[r0 + 32 : r0 + 33, :], start=True,
                                 stop=True, tile_position=(r0 + 32, 64))
                qn = p_tmp.tile([128, TB], BF16, tag="qn", name=f"qn_{name}")
                nc.vector.tensor_mul(qn[:], qb[:], pinvb[:])
                pqs = p_pq.tile([128, TB], F32, tag="pq", name=f"pqs_{name}")
                nc.tensor.matmul(pqs[:], pswap_sb[:], qn[:], start=True, stop=True)
                t1 = p_tmp.tile([128, TB], BF16, tag="t1", name=f"t1_{name}")
                nc.gpsimd.tensor_mul(t1[:], qn[:], cos_sb[:, jb])
                t2 = p_tmp.tile([128, TB], BF16, tag="t2", name=f"t2_{name}")
                nc.vector.tensor_mul(t2[:], pqs[:], sin_sb[:, jb])
                nc.gpsimd.tensor_add(dst[:, jb], t1[:], t2[:])

            def emit_v(tt):
                pv = p_pq.tile([128, TB], F32, tag="pq", name=f"pv{tt}")
                jc, tc_ = tt // 4, (tt % 4) * 128
                for ct in range(KT):
                    nc.tensor.matmul(
                        pv[:, 0:GF],
                        xt_sb[ct][jc][:, tc_ : tc_ + 128],
                        wv_fl[:, ct * GF : (ct + 1) * GF],
                        start=(ct == 0),
                        stop=(ct == KT - 1),
                    )
                v_t = p_v.tile([128, GH * (D + 1)], BF16, tag="v", name=f"v{tt}")
                vsrc = pv[:, 0:GF].rearrange("p (h d) -> p h d", h=GH)
                vdst = v_t[:].rearrange("p (h d) -> p h d", h=GH, d=D + 1)
                nc.vector.tensor_copy(vdst[:, :, 0:D], vsrc)
                nc.vector.memset(vdst[:, :, D : D + 1], 1.0)
                v_sb[tt] = v_t

            # ---- filler machinery ---------------------------------------
            fill_q = []

            def fill(n=1):
                for _ in range(n):
                    if not fill_q:
                        return
                    fill_q.pop(0)()

            def drain_fill():
                while fill_q:
                    fill_q.pop(0)()

            # ---- head: q/k mt0 projections + paired chains, v0..3 -------
            # per j: proj q, proj k, paired rsqrt; chain_b lags one j so the
            # PE never sits directly behind the ACT/DVE chain.  v-tile pairs
            # land mid-head so their DVE casts are ready for attention j0.
            recs = {}
            vq = [0, 1, 2, 3]
            for j in range(ntb):
                pq_q = emit_proj(wq_fl, 0, j)
                pq_k = emit_proj(wk_fl, 0, j)
                recs[j] = chain_a2(pq_q, pq_k, f"h0_{j}")
                # chain_b(j-1) after proj(j): by the time the PE reaches its
                # matmuls the DVE rsqrt for j-1 is long done (no stall).
                if j > 0:
                    qb0, kb0, inv0 = recs[j - 1]
                    jb0 = slice((j - 1) * TB, j * TB)
                    chain_b(qb0, inv0, False, qh_sb[0], jb0, f"q0_{j-1}")
                    chain_b(kb0, inv0, True, kh_sb[0], jb0, f"k0_{j-1}")
                # v0-3 one iteration earlier than strictly needed: attention
                # j0's AV matmuls consume them ~35us in, and the displaced
                # proj(2)/proj(3) work is off the critical path.
                if j >= 1:
                    for _ in range(2):
                        if vq:
                            emit_v(vq.pop(0))
            qb0, kb0, inv0 = recs[ntb - 1]
            jb0 = slice((ntb - 1) * TB, ntb * TB)
            chain_b(qb0, inv0, False, qh_sb[0], jb0, f"q0_{ntb-1}")
            chain_b(kb0, inv0, True, kh_sb[0], jb0, f"k0_{ntb-1}")
            while vq:
                emit_v(vq.pop(0))

            # ---- filler for attention phase -----------------------------
            # order: mt1 paired projections+chains per j, then v per deadline
            def mk_proj_chain(j):
                st = {}

                def do_a():
                    pq = emit_proj(wq_fl, 1, j)
                    pk = emit_proj(wk_fl, 1, j)
                    st["r"] = chain_a2(pq, pk, f"h1_{j}")

                def do_bq():
                    chain_b(st["r"][0], st["r"][2], False, qh_sb[1],
                            slice(j * TB, (j + 1) * TB), f"q1_{j}")

                def do_bk():
                    chain_b(st["r"][1], st["r"][2], True, kh_sb[1],
                            slice(j * TB, (j + 1) * TB), f"k1_{j}")

                return do_a, do_bq, do_bk

            for j in range(ntb):
                do_a, do_bq, do_bk = mk_proj_chain(j)
                fill_q.extend([do_a, do_bq, do_bk])
                if j < 3:
                    for tt in range(4 * (j + 1), 4 * (j + 2)):
                        fill_q.append(lambda tt=tt: emit_v(tt))

            # ---- attention ----------------------------------------------
            def norm(hp, j, po, y_t):
                # ACT-free: ACT is the bottleneck engine during attention.
                jb = slice(j * TB, (j + 1) * TB)
                rec = []
                for hl in range(2):
                    dn = p_tmp.tile([1, TB], F32, tag=f"den{hl}", name=f"dn{hl}")
                    nc.vector.tensor_copy(dn[:], po[hl][64:65, :])
                    rf = p_tmp.tile([1, TB], F32, tag=f"recf{hl}", name=f"rf{hl}")
                    nc.vector.reciprocal_approx_fast(out=rf[:], in_=dn[:])
                    rc = p_tmp.tile([1, TB], BF16, tag=f"rec{hl}", name=f"rc{hl}")
                    nc.vector.tensor_copy(rc[:], rf[:])
                    rec.append(rc)
                pr = p_pq.tile([128, TB], F32, tag="pq", name="pr")
                nc.tensor.matmul(
                    pr[0:64, :], ones64[0:1, :], rec[0][:], start=True, stop=True,
                    tile_position=(0, 0),
                )
                nc.tensor.matmul(
                    pr[64:128, :], ones64[0:1, :], rec[1][:], start=True, stop=True,
                    tile_position=(0, 64),
                )
                r_sb = p_tmp.tile([128, TB], BF16, tag="rsb", name="r_sb")
                nc.vector.tensor_copy(r_sb[:], pr[:])
                nc.vector.tensor_mul(y_t[0:64, jb], po[0][0:64, :], r_sb[0:64, :])
                nc.vector.tensor_mul(y_t[64:128, jb], po[1][0:64, :], r_sb[64:128, :])

            # partition-id-derived base for the valid A2A blocks (b = pid//4)
            pid = nc.sync.partition_id()
            base = nc.s_assert_within((pid >> 2) * 4, 0, 4, skip_runtime_assert=True)
            yg_sb = [None] * KT
            y_ts = []
            bounce_in = []
            bounce_out = []
            fill_ctr = 0
            for hp in range(2):
                y_t = p_y.tile([128, t], BF16, tag="y", name=f"y{hp}")
                y_ts.append(y_t)
                # exchange payload in fp8 (o_proj re-averages over 1024
                # inputs, so the 3-bit mantissa noise washes out) and the
                # bounce DMA for token block j fires as soon as its norm
                # lands, off the critical path to the collective trigger.
                bin_t = p_dram.tile([8 * 128, tsl], BF16, tag=f"bin{hp}")
                bout_t = p_dram.tile([8 * 128, tsl], BF16, tag=f"bout{hp}")
                bounce_in.append(bin_t)
                bounce_out.append(bout_t)
                for j in range(ntb):
                    jb = slice(j * TB, (j + 1) * TB)
                    po = [
                        p_po.tile([D + 1, TB], F32, tag="po", name=f"po{hl}")
                        for hl in range(2)
                    ]
                    n_tt = 4 * (j + 1)
                    pend = []
                    for tt in range(n_tt):
                        # scores for both heads -> one 2-bank psum tile
                        sc = p_sc.tile([128, 2 * TB], F32, tag="sc", name="sc")
                        for hl in range(2):
                            hofs = hl * 64
                            nc.tensor.matmul(
                                sc[:, hl * TB : (hl + 1) * TB],
                                kh_sb[hp][hofs : hofs + 64, tt * 128 : (tt + 1) * 128],
                                qh_sb[hp][hofs : hofs + 64, jb],
                                start=True,
                                stop=True,
                                tile_position=(hofs, 0),
                            )
                        pt = p_pt.tile([128, 2 * TB], BF16, tag="pt", name="pt")
                        nc.scalar.activation(
                            pt[:], sc[:], AF.Exp, scale=1.0 / np.sqrt(D)
                        )
                        r = tt - 4 * j
                        if r >= 0:  # diagonal tile: causal mask, both heads.
                            # only cols < 128*(r+1) of each head half can be
                            # masked; trim the mul to that range.
                            w = 128 * (r + 1)
                            pt3 = pt[:].rearrange("p (h c) -> p h c", h=2)
                            mk3 = mask_sb[
                                :, r * 2 * TB : (r + 1) * 2 * TB
                            ].rearrange("p (h c) -> p h c", h=2)
                            nc.vector.tensor_mul(
                                pt3[:, :, 0:w], pt3[:, :, 0:w], mk3[:, :, 0:w]
                            )
                        pend.append((tt, pt))
                        if len(pend) > 2:
                            att_tt, att_pt = pend.pop(0)
                            for hl in range(2):
                                h = 2 * hp + hl
                                nc.tensor.matmul(
                                    po[hl][:],
                                    v_sb[att_tt][:, h * (D + 1) : (h + 1) * (D + 1)],
                                    att_pt[:, hl * TB : (hl + 1) * TB],
                                    start=(att_tt == 0),
                                    stop=(att_tt == n_tt - 1),
                                )
                        # pace fillers ~2-per-3 steps through hp0 so PE work
                        # remains for the late j blocks (an exhausted queue
                        # left j3-boundary gaps that re-throttle the HAM);
                        # deadlines still hold: each v tile lands >=4 steps
                        # before its first attention consumer.
                        if hp == 1:
                            fill(1)
                        else:
                            fill_ctr += 1
                            if fill_ctr % 3 != 0:
                                fill(1)
                    for att_tt, att_pt in pend:
                        for hl in range(2):
                            h = 2 * hp + hl
                            nc.tensor.matmul(
                                po[hl][:],
                                v_sb[att_tt][:, h * (D + 1) : (h + 1) * (D + 1)],
                                att_pt[:, hl * TB : (hl + 1) * TB],
                                start=(att_tt == 0),
                                stop=(att_tt == n_tt - 1),
                            )
                    norm(hp, j, po, y_t)
                    for s in (j, j + 4):
                        eng = nc.sync if s < 4 else nc.scalar
                        eng.dma_start(
                            bin_t[s * 128 : (s + 1) * 128, :],
                            y_t[:, j * tsl : (j + 1) * tsl],
                        )

                # 8-way AllToAll (4-core groups unsupported): shard s carries
                # our features(hp) for token block s%4.
                nc.gpsimd.collective_compute(
                    "AllToAll",
                    mybir.AluOpType.bypass,
                    ins=[bin_t.opt()],
                    outs=[bout_t.opt()],
                    replica_groups=[[0, 1, 2, 3, 4, 5, 6, 7]],
                )
                for gi in range(4):
                    yg_t = p_yg.tile([128, tsl], BF16, tag="yg", name=f"yg{hp}_{gi}")
                    nc.sync.dma_start(
                        yg_t[:], bout_t[bass.ts(base + gi, 128), :]
                    )
                    yg_sb[2 * gi + hp] = yg_t

            drain_fill()

            # ---- gather valid A2A blocks (dynamic: depends on batch) ----
            # our batch's ranks are 4b..4b+3 where b = partition_id // 4.
            # yg[2*gi + hp] = features [256*gi + 128*hp, +128) of our tokens.
            tc.tile_set_cur_wait(50)  # push o_proj to the schedule end

            # ---- o_proj: out^T[cout, tsl], junk-free 8-mt contraction ----
            # mt-outer so each phase starts the moment its first yg block
            # lands.  Even mts (hp0, from AllToAll #1) accumulate during the
            # second AllToAll; a keep-warm chain paces the PE through the
            # collective so the odd-mt phase runs at full clock.  co7 runs
            # as one deferred full contraction, freeing its PSUM half-bank
            # as the keep-warm target.
            def wo_sl(mt, co):
                return wo_fl[:, mt * C + co * 128 : mt * C + co * 128 + 128]

            pouts = []
            for co in range(6):
                if co < 2:
                    pout = p_po.tile([128, TB], F32, tag="po", name=f"oo{co}")
                elif co < 4:
                    pout = p_pq.tile([128, TB], F32, tag="pq", name=f"oo{co}")
                elif co == 4:
                    sc2 = p_sc.tile([128, 2 * TB], F32, tag="sc", name=f"oo{co}")
                    pout = sc2[:, 0:TB]
                else:
                    pout = sc2[:, TB : 2 * TB]
                pouts.append(pout)
            # dedicated keep-warm bank: sharing a tile with an open o_proj
            # accumulation would serialize the chain behind the stop matmul
            # (tile-granular dependency tracking).
            kw_tail = p_sc.tile([128, 2 * TB], F32, tag="sc", name="kw_tail")
            for n_mt, mt in enumerate((0, 2, 4, 6)):
                for co in range(6):
                    nc.tensor.matmul(
                        pouts[co][:], wo_sl(mt, co), yg_sb[mt][:],
                        start=(n_mt == 0), stop=False,
                    )
            tc.tile_set_cur_wait(51)  # keep-warm strictly between the phases
            emit_keepwarm(
                20, kw_tail, "t", y_ts[1][0:1, (ntb - 1) * TB : ntb * TB]
            )
            tc.tile_set_cur_wait(52)

            def store(co, pout):
                o_sb = p_tmp.tile([128, tsl], F32, tag=f"osb{co % 2}", name="o_sb")
                if co % 2 == 0:
                    nc.vector.tensor_copy(o_sb[:], pout[:])
                else:
                    nc.scalar.copy(o_sb[:], pout[:])
                deng = nc.sync if co % 2 == 0 else nc.scalar
                deng.dma_start(out[co * 128 : (co + 1) * 128, :], o_sb[:])

            # co-outer so each co's store fires the moment its contraction
            # stops; co6/co7 (deferred full contractions -- their banks
            # hosted the keep-warm chain) land on the po/pq rings, which
            # free first (right after co0/co2's stores).
            for co in range(6):
                for n_mt, mt in enumerate((1, 3, 5, 7)):
                    nc.tensor.matmul(
                        pouts[co][:], wo_sl(mt, co), yg_sb[mt][:],
                        start=False, stop=(n_mt == 3),
                    )
                store(co, pouts[co])
            for co in (6, 7):
                pool = p_po if co == 6 else p_pq
                pd = pool.tile([128, TB], F32, tag="po" if co == 6 else "pq",
                               name=f"oo{co}")
                for mt in range(KT):
                    nc.tensor.matmul(
                        pd[:], wo_sl(mt, co), yg_sb[mt][:],
                        start=(mt == 0), stop=(mt == KT - 1),
                    )
                store(co, pd)

    nc.compile()
    return nc


# ---------------------------------------------------------------------------
# host side
# ---------------------------------------------------------------------------


def _rope_tables(t):
    inv_freq = 1.0 / (ROPE_BASE ** (np.arange(0, D, 2, dtype=np.float64) / D))  # [32]
    ang = np.arange(t, dtype=np.float64)[:, None] * inv_freq[None, :]  # [t, 32]
    cos = np.cos(ang).astype(np.float32)
    sin = np.sin(ang).astype(np.float32)
    cosf = np.empty((128, t), np.float32)
    sinf = np.empty((128, t), np.float32)
    for r in range(128):
        d = r % 64
        f = d if d < 32 else d - 32
        cosf[r] = cos[:, f]
        sinf[r] = -sin[:, f] if d < 32 else sin[:, f]
    return cosf, sinf


def _consts(t):
    cosf, sinf = _rope_tables(t)
    pswap = np.zeros((128, 128), np.float32)
    for j in range(128):
        d = j % 64
        i = (j - 32) if d >= 32 else (j + 32)
        pswap[i, j] = 1.0
    maskt = np.zeros((128, 4 * 2 * TB), np.float32)
    for r in range(4):
        for p in range(128):
            lo = 128 * r + p
            if lo < TB:
                maskt[p, r * 2 * TB + lo : r * 2 * TB + TB] = 1.0
                maskt[p, r * 2 * TB + TB + lo : (r + 1) * 2 * TB] = 1.0
    return cosf, sinf, pswap, maskt


def _blk2w(w):
    # blk2 with 1/w^2 weights: pss then sums (w*q)^2 / w^2 = q^2, so the
    # rsqrt sees the unscaled mean-square while Wq rows carry the w factor.
    wf = np.where(w == 0.0, 1.0, w.astype(np.float64))
    e = np.zeros((128, 2), np.float32)
    e[0:64, 0] = 1.0 / wf[0:64] ** 2
    e[64:128, 1] = 1.0 / (wf[0:64] if len(w) == 64 else wf[64:128]) ** 2
    return e


def _bf(x):
    return np.ascontiguousarray(x).astype(ml_dtypes.bfloat16)


def make_in_maps(x, Wq, Wk, Wv, Wo, qn_w, kn_w, t=T):
    cosf, sinf, pswap, maskt = _consts(t)
    wscale_q = np.tile(np.asarray(qn_w, np.float32), H)
    wscale_k = np.tile(np.asarray(kn_w, np.float32), H)
    Wq = Wq * wscale_q[:, None]
    Wk = Wk * wscale_k[:, None]
    common = {
        "cosf": _bf(cosf),
        "sinf": _bf(sinf),
        "pswap": _bf(pswap),
        "blk2q": _bf(_blk2w(qn_w)),
        "blk2k": _bf(_blk2w(kn_w)),
    }
    wot = _bf(Wo.T)  # [c_in, c_out]
    in_maps = []
    for c in range(N_CORES):
        b, g = c // 4, c % 4
        fs = slice(GF * g, GF * (g + 1))
        in_maps.append(
            dict(
                common,
                xt=_bf(x[b, :t, :].T),
                wq=_bf(Wq[fs, :].T),
                wk=_bf(Wk[fs, :].T),
                wv=_bf(Wv[fs, :].T),
                wo=wot,
            )
        )
    return in_maps


def assemble(results, t=T):
    tsl = t // 4
    out = np.empty((B, t, C), np.float32)
    for c in range(N_CORES):
        b, g = c // 4, c % 4
        out[b, g * tsl : (g + 1) * tsl, :] = results[c]["out"].T
    return out


# -- cached PJRT runner (compile once, reuse across kernel() calls) ---------

_RUNNER = {}


def _get_runner(t=T):
    if t in _RUNNER:
        return _RUNNER[t]
    import jax
    from jax.sharding import Mesh, PartitionSpec
    from jax.experimental.shard_map import shard_map
    from concourse import bass2jax

    nc = build_nc(t)
    bass2jax.install_neuronx_cc_hook()

    partition_name = nc.partition_id_tensor.name if nc.partition_id_tensor else None
    in_names = []
    out_names = []
    out_avals = []
    zero_outs = []
    for alloc in nc.m.functions[0].allocations:
        if not isinstance(alloc, mybir.MemoryLocationSet):
            continue
        name = alloc.memorylocations[0].name
        if alloc.kind == "ExternalInput":
            if name == partition_name:
                continue
            in_names.append(name)
        elif alloc.kind == "ExternalOutput":
            shape = tuple(alloc.tensor_shape)
            dtype = mybir.dt.np(alloc.dtype)
            out_names.append(name)
            out_avals.append(jax.core.ShapedArray(shape, dtype))
            zero_outs.append(np.zeros(shape, dtype))
    n_params = len(in_names)
    all_names = in_names + out_names
    if partition_name is not None:
        all_names = all_names + [partition_name]

    def _body(*args):
        operands = list(args)
        if partition_name is not None:
            operands.append(bass2jax.partition_id_tensor())
        outs = bass2jax._bass_exec_p.bind(
            *operands,
            out_avals=tuple(out_avals),
            in_names=tuple(all_names),
            out_names=tuple(out_names),
            lowering_input_output_aliases=(),
            sim_require_finite=True,
            sim_require_nnan=True,
            nc=nc,
        )
        return tuple(outs)

    devices = jax.devices()[:N_CORES]
    mesh = Mesh(np.asarray(devices), ("core",))
    fn = jax.jit(
        shard_map(
            _body,
            mesh=mesh,
            in_specs=(PartitionSpec("core"),) * (n_params + len(out_names)),
            out_specs=(PartitionSpec("core"),) * len(out_names),
            check_rep=False,
        ),
        keep_unused=True,
    )
    runner = {
        "fn": fn,
        "body": _body,
        "in_names": in_names,
        "out_names": out_names,
        "out_avals": out_avals,
        "zero_outs": zero_outs,
        "jax": jax,
    }
    _RUNNER[t] = runner
    return runner


def run_device(in_maps, t=T):
    r = _get_runner(t)
    concat_in = [
        np.concatenate([np.asarray(m[name]) for m in in_maps], axis=0)
        for name in r["in_names"]
    ]
    concat_zero = [
        np.zeros((N_CORES * z.shape[0], *z.shape[1:]), z.dtype) for z in r["zero_outs"]
    ]
    outs = r["fn"](*concat_in, *concat_zero)
    results = []
    for c in range(N_CORES):
        results.append(
            {
                name: np.asarray(outs[i]).reshape(N_CORES, *r["out_avals"][i].shape)[c]
                for i, name in enumerate(r["out_names"])
            }
        )
    return results


def kernel(x, Wq, Wk, Wv, Wo, qn_w, kn_w):
    x = np.asarray(x, np.float32)
    in_maps = make_in_maps(
        x,
        np.asarray(Wq, np.float32),
        np.asarray(Wk, np.float32),
        np.asarray(Wv, np.float32),
        np.asarray(Wo, np.float32),
        np.asarray(qn_w, np.float32),
        np.asarray(kn_w, np.float32),
    )
    results = run_device(in_maps)
    return assemble(results)

